# revision 25
# baseline (speedup 1.0000x reference)
"""Trainium2 Bass kernel for nn_Attention_57406532878693 (pooling attention).

Math (per (b, h) slice; T=2048, N=128, K2=16):
    x      = hyp[:, b, h*128:(h+1)*128]                    # (T, N)
    m      = x.mean(0)                                     # (N,)
    gx     = tanh(x @ W_w.T + W_b)                         # (T, K2)
    gm     = tanh(Wm_w @ m + Wm_b)                         # (K2,)
    u      = Wh_w[0] * gm                                  # (K2,)
    l      = gx @ u + Wh_b                                 # (T,)
    p      = exp(l)          (no max-sub needed: |l| <= 4.25, tanh-bounded)
    c      = (p @ x) / p.sum()                             # (N,)
    out[b, h*128:(h+1)*128] = c

Sharding: data-parallel over B across 8 cores (4 batches per core).

v6 design (vs v4's 145us harness / 148.7us TimelineSim):
  - psB (the 128 accumulating mean matmuls, ~27us of PE) is gone: the
    PSUM->SBUF transpose evacuations carry accum_out, producing per-
    (head, j-block) partial t-sums for free; a tiny DVE fold + 4
    single-column matmuls reconstruct Wm@mean.
  - the 8 (batch, head-quad) slots are software-pipelined: slot k's
    emission order is [transposes+evac, psA+tanh, meanMM], psW(k-1),
    [tanhc, u4, logits, exp, z, recip], load(k+2).  psW of the previous
    slot fills the PE while slot k's gate chain round-trips through
    ACT/DVE, which removes the per-slot PE stalls and shortens the tail.
  - consts ride in 2 packed DMAs (one f32, one bf16); nat loads are
    per-hq (4 tiles) with a 2-slot prefetch ring (12 tiles live max);
    outputs DMA out per batch instead of once at the end.
TimelineSim: 148.7us (v4) -> 127.4us (psB cut) -> see below.
Numerics: rel err ~2e-4 vs reference (bf16 gate path; f32r wsum).
"""

import os
import numpy as np

T, B, D = 2048, 32, 1024
H, N, K2 = 8, 128, 16
NCORES = 8
BL = B // NCORES          # 4 batches per core
TC = T // 128             # 16 t-chunks of 128
NQ = 4                    # nat tiles per (batch, head-quad); each holds 512 t
QW = 4 * N                # 512 cols per head-quad

LAST_RESULT = {}          # exec_time_ns etc. for test harness introspection


def _build(nc, tile, mybir, bass, whb_val, repeat=1, loop_n=0):
    f32 = mybir.dt.float32
    f32r = mybir.dt.float32r
    bf16 = mybir.dt.bfloat16
    AF = mybir.ActivationFunctionType

    hyp_s = nc.dram_tensor("hyp_s", [T, BL, D], f32, kind="ExternalInput").ap()
    cpack_d = nc.dram_tensor("cpack", [128, 136], f32,
                             kind="ExternalInput").ap()
    wgzm_d = nc.dram_tensor("wgzm", [N, 64], bf16, kind="ExternalInput").ap()
    out_s = nc.dram_tensor("out_s", [BL, D], f32, kind="ExternalOutput").ap()

    def r(ap):
        return ap.bitcast(f32r)

    with tile.TileContext(nc) as tc:
        from contextlib import ExitStack

        with ExitStack() as ctx:
            natf_b = int(os.environ.get("KB_NATF", "14"))
            xt_b = int(os.environ.get("KB_XT", "8"))
            g_b = int(os.environ.get("KB_G", "4"))
            pst_b = int(os.environ.get("KB_PST", "3"))
            psa_b = int(os.environ.get("KB_PSA", "2"))
            psc_b = int(os.environ.get("KB_PSC", "2"))
            psw_b = int(os.environ.get("KB_PSW", "1"))
            # number of the 16 per-slot PSUM->SBUF evacuations routed to
            # DVE (the rest go to ACT, which also carries tanh/exp/out)
            evdve = int(os.environ.get("KB_EVDVE", "11"))
            evdve_tail = int(os.environ.get("KB_EVDVET", "16"))
            cpool = ctx.enter_context(tc.tile_pool(name="consts", bufs=1))
            natf_pool = ctx.enter_context(tc.tile_pool(name="natf", bufs=natf_b))
            xt_pool = ctx.enter_context(tc.tile_pool(name="xt", bufs=xt_b))
            g_pool = ctx.enter_context(tc.tile_pool(name="g", bufs=g_b))
            sm_pool = ctx.enter_context(tc.tile_pool(name="small", bufs=4))
            out_pool = ctx.enter_context(tc.tile_pool(name="outp", bufs=1))
            pst_pool = ctx.enter_context(
                tc.tile_pool(name="pst", bufs=pst_b, space="PSUM"))
            psa_pool = ctx.enter_context(
                tc.tile_pool(name="psa", bufs=psa_b, space="PSUM"))
            psc_pool = ctx.enter_context(
                tc.tile_pool(name="psc", bufs=psc_b, space="PSUM"))
            psw_pool = ctx.enter_context(
                tc.tile_pool(name="psw", bufs=psw_b, space="PSUM"))

            cp = cpool.tile([128, 136], f32, tag="cpack")
            nc.sync.dma_start(r(cp[:]), r(cpack_d))
            wgzm = cpool.tile([N, 64], bf16, tag="wgzm")
            nc.sync.dma_start(wgzm[:], wgzm_d)
            ident = cp[:, 0:128]
            wbc = cp[:, 128:129]
            wmbc = cp[:, 129:130]
            whwm = cp[:, 130:134]
            ones_c = cp[:, 134:136]
            wgz = wgzm[:, 0:32]
            wmz = wgzm[:, 32:64]
            whb_c = cpool.tile([128, 1], f32, tag="whb")
            nc.gpsimd.memset(whb_c[:], float(whb_val))

            # PE transposes don't register as PE-busy for the HAM clock
            # governor, so a transpose-heavy stretch can drop the PE to
            # 1.2GHz.  KB_WARM sprinkles tiny real matmuls to keep the
            # activity window fed.
            warm = os.environ.get("KB_WARM", "1") == "1"
            warm_n = int(os.environ.get("KB_WARMN", "14"))
            wide_early = int(os.environ.get("KB_WIDE", "2"))
            # scratch region in the psw bank for warm matmuls: partition 0,
            # cols 256:512 — disjoint from every outscale read slice
            # ([32q, 128q:128(q+1)]), and strictly PE-ordered vs psW writes.
            warm_ps = psw_pool.tile([128, 512], f32, tag="psw")

            def ham_warm(width=2):
                if warm:
                    nc.tensor.matmul(warm_ps[0:1, 256:256 + width],
                                     r(ones_c[:, 0:1]),
                                     r(ident[0:128, 0:width]),
                                     start=True, stop=True,
                                     skip_group_check=True)

            if warm:
                # pre-loop clock warm-up: chained dummies on a never-written
                # scratch tile (contents irrelevant), so they have NO input
                # dependencies and start right after the preamble barrier —
                # the PE is at full clock by the time the first nat tile
                # lands.
                wsc = cpool.tile([128, 128], f32, tag="wsc")
                nc.gpsimd.memset(wsc[:], 1.0)
                for _ in range(warm_n):
                    nc.tensor.matmul(warm_ps[0:1, 0:128],
                                     r(wsc[:, 0:1]), r(wsc[:]),
                                     start=True, stop=True,
                                     skip_group_check=True)

            out_sb = out_pool.tile([97, BL * D // 4], f32, tag="out")

            def load_nat_hq(b, hq):
                tiles = []
                for j in range(NQ):
                    t0 = j * 4 * 128
                    nt = natf_pool.tile([128, 4 * QW], f32, tag="natf")
                    src = hyp_s[t0:t0 + 4 * 128, b:b + 1,
                                hq * QW:(hq + 1) * QW].rearrange(
                        "(c p) one d -> p c (one d)", p=128)
                    nc.sync.dma_start(
                        r(nt[:].rearrange("p (c d) -> p c d", c=4)),
                        r(src))
                    tiles.append(nt)
                return tiles

            def phase1a(natf, ev_n, wide=2):
                """transposes+evac (with mean accum), psA+tanh, mean MMs."""
                psC = psc_pool.tile([128, 72], f32, tag="psc")
                msum = sm_pool.tile([128, 16], f32, tag="msum")
                xts = []
                ei = 0
                for j in range(NQ):
                    nf = natf[j]
                    xt = xt_pool.tile([128, 4 * QW], bf16, tag="xt")
                    for q in range(4):
                        psT = pst_pool.tile([128, 512], f32, tag="pst")
                        ham_warm(wide)
                        for c in range(4):
                            nc.tensor.transpose(
                                r(psT[:, 128 * c:128 * (c + 1)]),
                                r(nf[:, 512 * c + 128 * q:
                                     512 * c + 128 * (q + 1)]),
                                r(ident[:]))
                        dst = xt[:, 512 * q:512 * (q + 1)]
                        acc = msum[:, 4 * q + j:4 * q + j + 1]
                        use_dve = (((ei + 1) * ev_n) // 16
                                   > (ei * ev_n) // 16)
                        ei += 1
                        if use_dve:
                            nc.vector.tensor_scalar(
                                dst, psT[:], 1.0, 0.0,
                                op0=mybir.AluOpType.mult,
                                op1=mybir.AluOpType.add,
                                accum_out=acc)
                        else:
                            nc.scalar.activation(dst, psT[:], AF.Copy,
                                                 accum_out=acc)
                    xts.append(xt)

                g_sb = g_pool.tile([128, T], bf16, tag="g")
                for j in range(NQ):
                    psA = psa_pool.tile([128, 512], f32, tag="psa")
                    for q in range(4):
                        rhs = xts[j][:, 512 * q:512 * (q + 1)]
                        nc.tensor.matmul(
                            psA[32 * q:32 * q + 32, :], wgz[:], rhs,
                            start=True, stop=True,
                            tile_position=(0, 32 * q),
                            skip_group_check=True)
                    nc.scalar.activation(
                        g_sb[:, 512 * j:512 * (j + 1)], psA[:],
                        AF.Tanh, bias=wbc[:])

                # mean-gate path: fold the 4 j-partials per head, then 4
                # single-column matmuls put Wm@mean at partitions 32q+k
                # of psC[:, 68]
                m4 = sm_pool.tile([128, 4], bf16, tag="m4")
                with nc.allow_low_precision(
                        reason="4-elem j-partial fold; |sums|~45, bf16 ok"):
                    nc.vector.tensor_reduce(
                        m4[:], msum[:].rearrange("p (q j) -> p q j", q=4),
                        axis=mybir.AxisListType.X,
                        op=mybir.AluOpType.add)
                for q in range(4):
                    nc.tensor.matmul(
                        psC[32 * q:32 * q + 32, 68:69], wmz[:],
                        m4[:, q:q + 1],
                        start=True, stop=True,
                        tile_position=(0, 32 * q),
                        skip_group_check=True)
                return {"psC": psC, "g_sb": g_sb}

            def zfin(st):
                """z matmul + reciprocal (deferred to the next iteration so
                it never head-of-line blocks the next slot's transposes)."""
                psC, pr_quad = st["psC"], st["pr_quad"]
                z_ps = psC[0:97, 64:66]
                nc.tensor.matmul(z_ps, r(pr_quad[:]), r(ones_c[:]),
                                 start=True, stop=True,
                                 skip_group_check=True)
                zi_sb = sm_pool.tile([97, 1], f32, tag="zi_sb")
                nc.vector.reciprocal(zi_sb[:], z_ps[0:97, 0:1])
                st["zi_sb"] = zi_sb

            def phase1b(st):
                """tanhc, u4, logits, memsets, exp."""
                psC, g_sb = st["psC"], st["g_sb"]
                tanhc = sm_pool.tile([128, 1], f32, tag="tanhc")
                nc.scalar.activation(tanhc[:], psC[:, 68:69], AF.Tanh,
                                     bias=wmbc[:])
                u4 = sm_pool.tile([128, 4], bf16, tag="u4")
                nc.vector.tensor_mul(
                    u4[:], whwm[:], tanhc[:].broadcast_to([128, 4]))

                # logits t-major: l[t, q] = sum_p g[p, t] * U4[p, q]
                for c in range(TC):
                    nc.tensor.matmul(
                        psC[:, 4 * c:4 * c + 4],
                        g_sb[:, 128 * c:128 * (c + 1)], u4[:],
                        start=True, stop=True, skip_group_check=True)

                p_quad = sm_pool.tile([128, 144], f32, tag="p_quad")
                pr_quad = sm_pool.tile([128, 97], f32, tag="pr_quad")
                nc.gpsimd.memset(p_quad[:], 0.0)
                nc.gpsimd.memset(pr_quad[:], 1.0)
                lview = psC[:, 0:64].rearrange("p (c q) -> p q c", q=4)
                with nc.allow_low_precision(
                        reason="f32r accum is fp32-width"):
                    for q in range(4):
                        nc.scalar.activation(
                            r(p_quad[:, 32 * q:32 * q + TC].unsqueeze(1)),
                            lview[:, q:q + 1, :],
                            AF.Exp, bias=whb_c[:],
                            accum_out=r(pr_quad[:, 32 * q:32 * q + 1]))
                st["p_quad"] = p_quad
                st["pr_quad"] = pr_quad

            def psw_mm(natf, st, c0, c1):
                """psW weighted-sum chunks [c0, c1)."""
                if c0 == 0:
                    psw_t = psw_pool.tile([128, 512], f32, tag="psw")
                    st["psW"] = psw_t
                psW, p_quad = st["psW"], st["p_quad"]
                for c in range(c0, c1):
                    j, cl = c // 4, c % 4
                    rhs = natf[j][:, 512 * cl:512 * (cl + 1)]
                    nc.tensor.matmul(psW[:], r(p_quad[:, c:c + 128]),
                                     r(rhs),
                                     start=(c == 0), stop=(c == TC - 1),
                                     skip_group_check=True)

            def phase2fin(b, hq, st):
                """out scale (+ per-batch out DMA)."""
                psW, zi_sb = st["psW"], st["zi_sb"]
                for q in range(4):
                    col = b * (D // 4) + hq * N
                    nc.vector.tensor_scalar(
                        out_sb[32 * q:32 * q + 1, col:col + N],
                        psW[32 * q:32 * q + 1, q * N:(q + 1) * N],
                        zi_sb[32 * q:32 * q + 1, 0:1], None,
                        op0=mybir.AluOpType.mult)
                if hq == 1:
                    # batch b fully scaled -> stream its row out now.  One
                    # partition-strided DMA, issued from the idle GPSIMD
                    # queue so it never blocks the SP load queue's head.
                    nc.gpsimd.dma_start(
                        out_s[b:b + 1, :].rearrange(
                            "one (j q n) -> one q j n", q=4, n=N),
                        out_sb[0:97:32,
                               b * (D // 4):(b + 1) * (D // 4)]
                        .rearrange("p (j n) -> p j n", n=N))

            def run_schedule(slots):
                n = len(slots)
                nat = {0: load_nat_hq(*slots[0])}
                if n > 1:
                    nat[1] = load_nat_hq(*slots[1])
                prev = None
                for i, (b, hq) in enumerate(slots):
                    st = phase1a(nat[i],
                                 evdve_tail if i == n - 1 else evdve,
                                 wide=wide_early if i < 2 else 2)
                    if prev is not None:
                        pi, pst_ = prev
                        zfin(pst_)
                        psw_mm(nat[pi], pst_, 0, TC // 2)
                    phase1b(st)
                    if prev is not None:
                        pi, pst_ = prev
                        psw_mm(nat[pi], pst_, TC // 2, TC)
                        phase2fin(slots[pi][0], slots[pi][1], pst_)
                        del nat[pi]
                    prev = (i, st)
                    if i + 2 < n:
                        nat[i + 2] = load_nat_hq(*slots[i + 2])
                pi, pst_ = prev
                zfin(pst_)
                psw_mm(nat[pi], pst_, 0, TC)
                phase2fin(slots[pi][0], slots[pi][1], pst_)

            base_slots = [(b, hq) for b in range(BL) for hq in range(2)]
            if loop_n:
                with tc.For_i(0, loop_n, 1):
                    run_schedule(base_slots)
            else:
                run_schedule(base_slots * repeat)
    return nc


def _consts(inputs):
    import ml_dtypes
    W_w = np.asarray(inputs["W_w"], dtype=np.float32)      # (K2, N)
    W_b = np.asarray(inputs["W_b"], dtype=np.float32)      # (K2,)
    Wm_w = np.asarray(inputs["Wm_w"], dtype=np.float32)    # (K2, N)
    Wm_b = np.asarray(inputs["Wm_b"], dtype=np.float32)    # (K2,)
    Wh_w = np.asarray(inputs["Wh_w"], dtype=np.float32)    # (1, K2)

    bf = ml_dtypes.bfloat16
    wgz = np.zeros((N, 32), np.float32)
    wgz[:, 0:K2] = W_w.T
    wmz = np.zeros((N, 32), np.float32)
    wmz[:, 0:K2] = Wm_w.T / T
    wgzm = np.concatenate([wgz, wmz], axis=1).astype(bf)   # (N, 64) bf16

    cpack = np.zeros((128, 136), np.float32)
    cpack[:, 0:128] = np.eye(128, dtype=np.float32)
    for q in range(4):
        cpack[32 * q:32 * q + K2, 128] = W_b
        cpack[32 * q:32 * q + K2, 129] = Wm_b
        cpack[32 * q:32 * q + K2, 130 + q] = Wh_w[0]
    cpack[:, 134:136] = 1.0
    return {"cpack": cpack, "wgzm": wgzm}


def kernel(**inputs):
    import concourse.bass as bass
    import concourse.bacc as bacc
    import concourse.tile as tile
    import concourse.mybir as mybir
    from concourse import bass_utils

    hyp = np.ascontiguousarray(np.asarray(inputs["hyp"], dtype=np.float32))
    Wh_b = np.asarray(inputs["Wh_b"], dtype=np.float32)    # (1,)

    nc = bacc.Bacc("TRN2", target_bir_lowering=False, debug=False)
    _build(nc, tile, mybir, bass, float(Wh_b.reshape(-1)[0]))
    nc.compile()

    consts = _consts(inputs)
    in_maps = []
    for j in range(NCORES):
        m = {"hyp_s": np.ascontiguousarray(hyp[:, j * BL:(j + 1) * BL, :])}
        m.update(consts)
        in_maps.append(m)

    trace = os.environ.get("BASS_KERNEL_TRACE", "0") == "1"
    res = bass_utils.run_bass_kernel_spmd(
        nc, in_maps, core_ids=list(range(NCORES)), trace=trace)

    LAST_RESULT.clear()
    LAST_RESULT["exec_time_ns"] = res.exec_time_ns
    LAST_RESULT["trace"] = (res.instructions_and_trace[1]
                            if res.instructions_and_trace else None)
    LAST_RESULT["profile_json"] = res.profile_json

    out = np.concatenate([res.results[j]["out_s"] for j in range(NCORES)],
                         axis=0)
    return out.astype(np.float32)
